# revision 33
# baseline (speedup 1.0000x reference)
"""Trainium2 Bass kernel: single transformer block (MHA + FFN + 2xLN).

Sharding: data-parallel over tokens. 8 cores; cores 0-3 own batch 0,
cores 4-7 own batch 1; each core owns 1024 consecutive tokens of its
batch. QKV/FFN/LN are token-local; attention needs all K/V of the
batch, obtained with 3 pipelined combined K+V AllGathers over each
4-core group (V scattered first, K chunks trigger each gather).

v4 layout strategy: all-bf16 matmuls (PSUM fp32). Activations are kept
transposed ([feature, token]); weights (torch Linear [out,in]) are
transposed once on the PE array, all during phase A so attention owns
all 8 PSUM banks. Attention processes HEAD PAIRS: the two heads of a
pair live on partition halves 0-63 / 64-127, and their contract-64
score matmuls are interleaved so the PE executes them concurrently on
disjoint row-halves of the array (measured 2x: 117ns vs 435ns for a
lone contract-64 matmul). The exp (Act engine, [128,1024] PSUM tiles)
is then the attention bottleneck (~430us) and the PE has ~40% slack.
ctx matmuls append a ones column to V so the softmax denominator
falls out of the PSUM accumulation; normalization runs off the
critical path (DVE reciprocal_approx_fast + gpsimd partition
broadcast + DVE multiply). LayerNorm statistics use ones-vector
matmuls on the PE; FFN is software-pipelined (psh(i+1) emitted before
ps2(i)) so the PE never waits on Gelu.
"""

import os
import sys

for _p in (
    "/opt/trn_rl_repo",
    "/root/.axon_site",
    "/root/.axon_site/_ro/trn_rl_repo",
    "/root/.axon_site/_ro/pypackages",
):
    if os.path.isdir(_p) and _p not in sys.path:
        sys.path.append(_p)

import numpy as np

import concourse.bass as bass
import concourse.mybir as mybir
import concourse.tile as tile
from concourse import bacc
from concourse.bass_utils import run_bass_kernel_spmd
from concourse.masks import make_identity

F32 = mybir.dt.float32
BF = mybir.dt.bfloat16
AF = mybir.ActivationFunctionType
ALU = mybir.AluOpType

B, S, D = 2, 4096, 768
H, DK = 12, 64
DFF = 3072
NCORES = 8
GROUP = 4  # cores per batch
TOK = (B * S) // NCORES  # 1024 tokens per core
TCH = TOK // 128  # 8
DCH = D // 128  # 6
FCH = DFF // 128  # 24
KV = S  # kv length per batch
KCH = KV // 128  # 32
EPS = 1e-5
RG = [[0, 1, 2, 3], [4, 5, 6, 7]]

NG = 3  # pipelined sub-gathers (4 heads = 2 head-pairs each)
HPG = H // NG  # heads per sub-gather (4)
CPG = HPG // 2  # K.T 128-row chunks per sub-gather (2)
VW = 65  # V cols per head: 64 value cols + ones col for the denominator
KG_ELEMS = 128 * CPG * TOK  # bf16 elems of K.T per sub-gather
VG_ELEMS = TCH * 128 * (HPG * VW)  # bf16 elems of V per sub-gather


def _percol(tc, const, t_in, name, n):
    """1D [n*128] fp32 -> SBUF [128, n] (feature-chunked per-column)."""
    nc = tc.nc
    t = const.tile([128, n], F32, tag=f"pc_{name}", name=f"pc_{name}")
    nc.sync.dma_start(t[:], t_in[name].rearrange("(c p) -> p c", p=128))
    return t


def _emit_ln(tc, ps_bc, ps_st, sb_tmp, y, g_sb, beta_sb, out):
    """LayerNorm along the partition (feature) axis of y [128, DCH, TOK] bf16.

    Stats via PE ones-matmuls into a single [33, TOK] PSUM tile (row 0 =
    sum, row 32 = sum of squares), sqrt on Act + approx reciprocal on DVE,
    broadcasts via fp32 ones-column matmuls, apply via DVE + Act.
    """
    nc = tc.nc
    ones_p = tc._ones_p_bf
    ones_f = tc._ones_f32
    st = ps_st.tile([33, TOK], F32, tag="st", name="st")
    for q in range(TOK // 512):
        qs = slice(q * 512, (q + 1) * 512)
        for j in range(DCH):
            nc.tensor.matmul(
                st[0:1, qs], ones_p[:], y[:, j, qs],
                start=(j == 0), stop=(j == DCH - 1), skip_group_check=True,
            )
    for j in range(DCH):
        sq = sb_tmp.tile([128, TOK], BF, tag="lnsq", name="sq")
        nc.vector.tensor_tensor(sq[:], y[:, j, :], y[:, j, :], ALU.mult)
        for q in range(TOK // 512):
            qs = slice(q * 512, (q + 1) * 512)
            nc.tensor.matmul(
                st[32:33, qs], ones_p[:], sq[:, qs],
                start=(j == 0), stop=(j == DCH - 1), skip_group_check=True,
            )
    mu = sb_tmp.tile([1, TOK], F32, tag="lnmu", name="mu")
    var = sb_tmp.tile([1, TOK], F32, tag="lnvar", name="var")
    rs = sb_tmp.tile([1, TOK], F32, tag="lnrs", name="rs")
    brow = sb_tmp.tile([1, TOK], F32, tag="lnbrow", name="brow")
    mu2 = sb_tmp.tile([1, TOK], F32, tag="lnmu2", name="mu2")
    nc.vector.tensor_scalar_mul(mu[:], st[0:1, :], 1.0 / D)
    nc.vector.tensor_scalar_mul(var[:], st[32:33, :], 1.0 / D)
    nc.vector.tensor_tensor(mu2[:], mu[:], mu[:], ALU.mult)  # mu^2
    nc.vector.tensor_tensor(var[:], var[:], mu2[:], ALU.subtract)
    # sd = sqrt(var + eps) on Act, then rs = 1/sd on DVE (approx is fine
    # at this tolerance)
    nc.scalar.activation(var[:], var[:], AF.Sqrt, bias=tc._eps[:])
    nc.vector.reciprocal_approx_fast(rs[:], var[:])
    nc.vector.tensor_tensor(brow[:], mu[:], rs[:], ALU.mult)  # mu*rs
    bcA = ps_bc.tile([128, TOK], F32, tag="big", name="bcA")
    bcB = ps_bc.tile([128, TOK], F32, tag="big", name="bcB")
    for q in range(TOK // 512):
        qs = slice(q * 512, (q + 1) * 512)
        nc.tensor.matmul(bcA[:, qs], ones_f[:], rs[:, qs],
                         start=True, stop=True, skip_group_check=True)
        nc.tensor.matmul(bcB[:, qs], ones_f[:], brow[:, qs],
                         start=True, stop=True, skip_group_check=True)
    for j in range(DCH):
        t1 = sb_tmp.tile([128, TOK], F32, tag="lnt", name="t1")
        nc.vector.tensor_tensor(t1[:], y[:, j, :], bcA[:], ALU.mult)
        nc.vector.tensor_tensor(t1[:], t1[:], bcB[:], ALU.subtract)
        nc.scalar.activation(out[:, j, :], t1[:], AF.Identity,
                             bias=beta_sb[:, j : j + 1], scale=g_sb[:, j : j + 1])


def _emit_body(tc, t_in, t_out):
    nc = tc.nc
    dbg = {k[4:]: v for k, v in t_out.items() if k.startswith("dbg_")}

    def dump(name, sb_ap):
        if name in dbg:
            nc.sync.dma_start(dbg[name], sb_ap)

    x_ap = t_in["x_shard"]
    out_ap = t_out["out_shard"]

    from contextlib import ExitStack

    with tc.tile_pool(name="const", bufs=1) as const, \
         tc.tile_pool(name="dram", bufs=1, space="DRAM") as dram, \
         tc.tile_pool(name="pAct", bufs=1) as pAct:
        _pw_stack = ExitStack()
        pW = _pw_stack.enter_context(tc.tile_pool(name="pW", bufs=1))

        ident = const.tile([128, 128], F32)
        make_identity(nc, ident[:])
        ident_bf = const.tile([128, 128], BF)
        nc.vector.tensor_copy(ident_bf[:], ident[:])
        ones_bf_col = const.tile([128, 1], BF)
        nc.vector.memset(ones_bf_col[:], 1.0)
        ones_bf_row = const.tile([1, 128], BF)
        nc.vector.memset(ones_bf_row[:], 1.0)
        ones_f32 = const.tile([1, 128], F32)
        nc.vector.memset(ones_f32[:], 1.0)
        ones_bf_h = const.tile([128, H], BF)
        nc.vector.memset(ones_bf_h[:], 1.0)
        eps_sb = const.tile([1, 1], F32)
        nc.vector.memset(eps_sb[:], EPS)
        tc._ones_p_bf = ones_bf_col
        tc._ones_f32 = ones_f32
        tc._eps = eps_sb

        bq_sb = _percol(tc, const, t_in, "bq", DCH)
        bk_sb = _percol(tc, const, t_in, "bk", DCH)
        bo_sb = _percol(tc, const, t_in, "bo", DCH)
        b1_sb = _percol(tc, const, t_in, "b1", FCH)
        b2_sb = _percol(tc, const, t_in, "b2", DCH)
        g1_sb = _percol(tc, const, t_in, "g1", DCH)
        beta1_sb = _percol(tc, const, t_in, "beta1", DCH)
        g2_sb = _percol(tc, const, t_in, "g2", DCH)
        beta2_sb = _percol(tc, const, t_in, "beta2", DCH)
        bv_row32 = const.tile([1, D], F32)
        nc.sync.dma_start(bv_row32[:], t_in["bv"].unsqueeze(0))
        bv_row = const.tile([1, D], BF)
        nc.vector.tensor_copy(bv_row[:], bv_row32[:])

        # DRAM scratch for the combined K+V all-gathers (bf16)
        kv_ins = [dram.tile([KG_ELEMS + VG_ELEMS], BF, tag=f"kvi{g}",
                            name=f"kv_in{g}") for g in range(NG)]
        kv_outs = [dram.tile([GROUP, KG_ELEMS + VG_ELEMS], BF, tag=f"kvo{g}",
                             name=f"kv_out{g}") for g in range(NG)]

        # Big activation tiles (bf16), reused across phases via tags.
        xT = pAct.tile([128, DCH, TOK], BF, tag="slotA")    # A..C (residual 1)
        QT = pAct.tile([128, DCH, TOK], BF, tag="slotB")    # A..B
        woT = pW.tile([128, DCH, D], BF, tag="woT")
        w1T = pW.tile([128, DCH, DFF], BF, tag="w1T")
        w2T = pW.tile([128, FCH, D], BF, tag="w2T")

        # ---- Phase A: x transpose, V, K, Q projections, gathers, and ALL
        # ---- weight transposes (wo/w1/w2 included, so attention gets all
        # ---- 8 PSUM banks).
        with tc.tile_pool(name="pA", bufs=2) as pA, \
             tc.tile_pool(name="pA1", bufs=3) as pA1, \
             tc.tile_pool(name="ps_tp", bufs=2, space="PSUM") as ps_tp, \
             tc.tile_pool(name="ps_qk", bufs=2, space="PSUM") as ps_qk:

            def transpose_w_bf(w_ap, n_out_ch, dest_fn, col0=0):
                """dest_fn(i, j) <- w[i-chunk, j-chunk].T (bf16 out)."""
                for i in range(n_out_ch):
                    win = pA1.tile([128, DCH * 128], F32, tag="win",
                                   name="win")
                    nc.sync.dma_start(
                        win[:], w_ap[i * 128 : (i + 1) * 128,
                                     col0 : col0 + DCH * 128])
                    for j in range(DCH):
                        tp = ps_tp.tile([128, 128], F32, tag="tp", name="tp")
                        nc.tensor.transpose(
                            tp[:], win[:, j * 128 : (j + 1) * 128], ident[:])
                        nc.vector.tensor_copy(dest_fn(i, j), tp[:])

            # x -> xT (bf16)
            for t in range(TCH):
                xin = pA1.tile([128, D], F32, tag="xin", name="xin")
                nc.sync.dma_start(xin[:], x_ap[t * 128 : (t + 1) * 128, :])
                for j in range(DCH):
                    tp = ps_tp.tile([128, 128], F32, tag="tp", name="tp")
                    nc.tensor.transpose(tp[:], xin[:, j * 128 : (j + 1) * 128],
                                        ident[:])
                    nc.vector.tensor_copy(xT[:, j, t * 128 : (t + 1) * 128],
                                          tp[:])

            wT = {}

            def next_wT(wname):
                wT[wname] = pA.tile([128, DCH, D], BF, tag="wT",
                                    name=f"{wname}T")

            # V (natural layout [tok, dout] + per-head ones column)
            next_wT("wv")
            transpose_w_bf(t_in["wv"], DCH,
                           lambda i, j: wT["wv"][:, j, i * 128 : (i + 1) * 128])
            for t in range(TCH):
                psv = ps_qk.tile([128, TOK], F32, tag="qk", name="psv")
                for lo, hi in ((0, 512), (512, D)):
                    qs = slice(lo, hi)
                    for j in range(DCH):
                        nc.tensor.matmul(
                            psv[:, qs],
                            xT[:, j, t * 128 : (t + 1) * 128],
                            wT["wv"][:, j, qs],
                            start=(j == 0), stop=False,
                            skip_group_check=True,
                        )
                    nc.tensor.matmul(  # bias row: + ones.T @ bv
                        psv[:, qs], ones_bf_row[:], bv_row[0:1, qs],
                        start=False, stop=True, skip_group_check=True,
                    )
                vt = pA1.tile([128, H * VW], BF, tag="vtev", name="vt")
                vt_h = vt[:].rearrange("p (h f) -> p h f", h=H)
                nc.vector.tensor_copy(
                    vt_h[:, :, 0:DK],
                    psv[:, 0:D].rearrange("p (h f) -> p h f", h=H),
                )
                nc.vector.tensor_copy(
                    vt_h[:, :, DK : DK + 1], ones_bf_h[:].unsqueeze(2),
                )
                for g in range(NG):
                    nc.sync.dma_start(
                        kv_ins[g][KG_ELEMS:].rearrange(
                            "(t p f) -> t p f", t=TCH, p=128)[t],
                        vt[:, g * HPG * VW : (g + 1) * HPG * VW],
                    )

            # K: bf16 K.T chunks; combined sub-gather g launches after its
            # chunk pair (the V part of the region is already written).
            next_wT("wk")
            transpose_w_bf(t_in["wk"], DCH,
                           lambda i, j: wT["wk"][:, j, i * 128 : (i + 1) * 128])
            for m in range(DCH):
                pso = ps_qk.tile([128, TOK], F32, tag="qk", name="pso")
                for q in range(TOK // 512):
                    qs = slice(q * 512, (q + 1) * 512)
                    for j in range(DCH):
                        nc.tensor.matmul(
                            pso[:, qs],
                            wT["wk"][:, j, m * 128 : (m + 1) * 128],
                            xT[:, j, qs],
                            start=(j == 0), stop=(j == DCH - 1),
                            skip_group_check=True,
                        )
                kt = pA1.tile([128, TOK], BF, tag="ktev", name="kt")
                nc.scalar.activation(kt[:], pso[:], AF.Identity,
                                     bias=bk_sb[:, m : m + 1])
                g = m // CPG
                nc.sync.dma_start(
                    kv_ins[g][0:KG_ELEMS].rearrange(
                        "(p c t) -> p c t", p=128, c=CPG)[:, m % CPG, :],
                    kt[:],
                )
                if m % CPG == CPG - 1:
                    nc.gpsimd.collective_compute(
                        "AllGather", ALU.bypass, replica_groups=RG,
                        ins=[kv_ins[g][:].opt()],
                        outs=[kv_outs[g][:].opt()],
                    )

            # Q
            next_wT("wq")
            transpose_w_bf(t_in["wq"], DCH,
                           lambda i, j: wT["wq"][:, j, i * 128 : (i + 1) * 128])
            for m in range(DCH):
                pso = ps_qk.tile([128, TOK], F32, tag="qk", name="pso")
                for q in range(TOK // 512):
                    qs = slice(q * 512, (q + 1) * 512)
                    for j in range(DCH):
                        nc.tensor.matmul(
                            pso[:, qs],
                            wT["wq"][:, j, m * 128 : (m + 1) * 128],
                            xT[:, j, qs],
                            start=(j == 0), stop=(j == DCH - 1),
                            skip_group_check=True,
                        )
                nc.scalar.activation(QT[:, m, :], pso[:], AF.Identity,
                                     bias=bq_sb[:, m : m + 1])

            # wo/w1/w2 -> SBUF-resident bf16 transposed tiles (overlaps the
            # gather waits; attention then needs no PSUM for transposes)
            transpose_w_bf(t_in["wo"], DCH,
                           lambda i, j: woT[:, j, i * 128 : (i + 1) * 128])
            transpose_w_bf(t_in["w1"], FCH,
                           lambda i, j: w1T[:, j, i * 128 : (i + 1) * 128])
            for quarter in range(4):
                transpose_w_bf(
                    t_in["w2"], DCH,
                    lambda i, j, _q=quarter: w2T[:, _q * DCH + j,
                                                 i * 128 : (i + 1) * 128],
                    col0=quarter * D)
            dump("xT", xT[:])
            dump("QT", QT[:])

        # ---- Phase B: attention over head pairs ---------------------------
        ctxT = pAct.tile([128, DCH, TOK], BF, tag="slotC")  # B..C

        with tc.tile_pool(name="pB", bufs=2) as pB, \
             tc.tile_pool(name="pBe", bufs=4) as pBe, \
             tc.tile_pool(name="pBt", bufs=2) as pBt, \
             tc.tile_pool(name="ps_sc", bufs=2, space="PSUM") as ps_sc, \
             tc.tile_pool(name="ps_ce", bufs=1, space="PSUM") as ps_ce, \
             tc.tile_pool(name="ps_co", bufs=1, space="PSUM") as ps_co:

            for jch in range(DCH):  # head pair (2*jch, 2*jch+1)
                g = (2 * jch) // HPG
                cc = jch % CPG
                # K.T for both heads of the pair in one [128, KV] tile:
                # even head at partitions 0-63, odd at 64-127.
                KhT = pB.tile([128, KV], BF, tag="kh", name="KhT")
                Vhe = pB.tile([128, KCH, VW], BF, tag="vhe", name="Vhe")
                Vho = pB.tile([128, KCH, VW], BF, tag="vho", name="Vho")
                for r in range(GROUP):
                    nc.sync.dma_start(
                        KhT[:, r * TOK : (r + 1) * TOK],
                        kv_outs[g][r, 0:KG_ELEMS].rearrange(
                            "(p c t) -> p c t", p=128, c=CPG)[:, cc, :],
                    )
                    for hh, Vt in (((2 * jch) % HPG, Vhe),
                                   ((2 * jch + 1) % HPG, Vho)):
                        nc.sync.dma_start(
                            Vt[:, r * TCH : (r + 1) * TCH, :],
                            kv_outs[g][r, KG_ELEMS:].rearrange(
                                "(t p f) -> p t f", t=TCH, p=128
                            )[:, :, hh * VW : (hh + 1) * VW],
                        )
                if jch == 0:
                    dump("KhT0", KhT[:])
                    dump("Vh0", Vhe[:])
                acc_e = ps_ce.tile([VW, TOK], F32, tag="ce", name="acc_e")
                acc_o = ps_co.tile([VW, TOK], F32, tag="co", name="acc_o")
                # Software-pipelined: scores+exp for chunk c stream ahead;
                # ctx for chunk c-1 is emitted after exp(c). The even/odd
                # score matmuls are interleaved so they run concurrently on
                # the two row-halves of the PE array.
                Ee_prev = Eo_prev = None
                for c in range(KCH):
                    ps_se = ps_sc.tile([128, TOK], F32, tag="s", name="ps_se")
                    ps_so = ps_sc.tile([128, TOK], F32, tag="s", name="ps_so")
                    for q in range(TOK // 512):
                        qs = slice(q * 512, (q + 1) * 512)
                        nc.tensor.matmul(
                            ps_se[:, qs],
                            KhT[0:64, c * 128 : (c + 1) * 128],
                            QT[0:64, jch, qs],
                            start=True, stop=True, skip_group_check=True,
                        )
                        nc.tensor.matmul(
                            ps_so[:, qs],
                            KhT[64:128, c * 128 : (c + 1) * 128],
                            QT[64:128, jch, qs],
                            start=True, stop=True, skip_group_check=True,
                        )
                    Ee = pBe.tile([128, TOK], BF, tag="E", name="Ee")
                    nc.scalar.activation(Ee[:], ps_se[:], AF.Exp,
                                         scale=1.0 / float(np.sqrt(DK)))
                    Eo = pBe.tile([128, TOK], BF, tag="E", name="Eo")
                    nc.scalar.activation(Eo[:], ps_so[:], AF.Exp,
                                         scale=1.0 / float(np.sqrt(DK)))
                    if jch == 0 and c == 0:
                        dump("E0", Ee[:])
                    if Ee_prev is not None:
                        for q in range(TOK // 512):
                            qs = slice(q * 512, (q + 1) * 512)
                            nc.tensor.matmul(
                                acc_e[:, qs], Vhe[:, c - 1, :], Ee_prev[:, qs],
                                start=(c - 1 == 0), stop=False,
                                skip_group_check=True,
                            )
                            nc.tensor.matmul(
                                acc_o[:, qs], Vho[:, c - 1, :], Eo_prev[:, qs],
                                start=(c - 1 == 0), stop=False,
                                skip_group_check=True,
                            )
                    Ee_prev, Eo_prev = Ee, Eo
                for q in range(TOK // 512):
                    qs = slice(q * 512, (q + 1) * 512)
                    nc.tensor.matmul(
                        acc_e[:, qs], Vhe[:, KCH - 1, :], Ee_prev[:, qs],
                        start=False, stop=True, skip_group_check=True,
                    )
                    nc.tensor.matmul(
                        acc_o[:, qs], Vho[:, KCH - 1, :], Eo_prev[:, qs],
                        start=False, stop=True, skip_group_check=True,
                    )
                # normalize both heads off the critical path
                for plo, acc in ((0, acc_e), (64, acc_o)):
                    den = pBt.tile([1, TOK], F32, tag="den", name="den")
                    nc.vector.tensor_copy(den[:], acc[64:65, :])
                    rec = pBt.tile([1, TOK], F32, tag="rec", name="rec")
                    nc.vector.reciprocal_approx_fast(rec[:], den[:])
                    bcr = pBt.tile([64, TOK], F32, tag="bcr", name="bcr")
                    nc.gpsimd.partition_broadcast(bcr[:], rec[:])
                    nc.vector.tensor_tensor(
                        ctxT[plo : plo + 64, jch, :], acc[0:64, :],
                        bcr[:], ALU.mult,
                    )

        dump("ctxT", ctxT[:])

        # ---- Phase C: O-projection + residual + LN1 -----------------------
        n1 = pAct.tile([128, DCH, TOK], BF, tag="slotB")

        with tc.tile_pool(name="pC2", bufs=2) as pC2, \
             tc.tile_pool(name="ps_o", bufs=2, space="PSUM") as ps_o, \
             tc.tile_pool(name="ps_st", bufs=1, space="PSUM") as ps_st:
            y1 = pAct.tile([128, DCH, TOK], BF, tag="slotD", name="y1")
            for m in range(DCH):
                pso = ps_o.tile([128, TOK], F32, tag="big", name="pso")
                for q in range(TOK // 512):
                    qs = slice(q * 512, (q + 1) * 512)
                    for j in range(DCH):
                        nc.tensor.matmul(
                            pso[:, qs],
                            woT[:, j, m * 128 : (m + 1) * 128],
                            ctxT[:, j, qs],
                            start=(j == 0), stop=(j == DCH - 1),
                            skip_group_check=True,
                        )
                # y1 = (pso + bo) + x  (fused on DVE)
                nc.vector.scalar_tensor_tensor(
                    y1[:, m, :], pso[:], bo_sb[:, m : m + 1], xT[:, m, :],
                    ALU.add, ALU.add,
                )
            dump("y1", y1[:])
            _emit_ln(tc, ps_o, ps_st, pC2, y1, g1_sb, beta1_sb, n1)
            dump("n1", n1[:])

        # =================== Phase D: FFN (+ residual) =====================
        y2 = pAct.tile([128, DCH, TOK], BF, tag="slotA")  # reuses xT slot
        with tc.tile_pool(name="ps_f2", bufs=1, space="PSUM") as ps_f2, \
             tc.tile_pool(name="ps_h", bufs=2, space="PSUM") as ps_h, \
             tc.tile_pool(name="pDh", bufs=3) as pDh:
            for half in range(2):
                hs = slice(half * 512, (half + 1) * 512)
                ps2 = ps_f2.tile([128, DCH, 512], F32, tag="ffn2", name="ps2")

                def emit_psh(i):
                    psh = ps_h.tile([128, 512], F32, tag="h", name="psh")
                    for j in range(DCH):
                        nc.tensor.matmul(
                            psh[:], w1T[:, j, i * 128 : (i + 1) * 128],
                            n1[:, j, hs],
                            start=(j == 0), stop=(j == DCH - 1),
                            skip_group_check=True,
                        )
                    return psh

                # Software-pipelined: psh(i+1) is emitted before ps2(i) so
                # the PE works while Gelu(i) runs on Act.
                psh = emit_psh(0)
                for i in range(FCH):
                    hsb = pDh.tile([128, 512], BF, tag="hsb", name="hsb")
                    nc.scalar.activation(hsb[:], psh[:], AF.Gelu,
                                         bias=b1_sb[:, i : i + 1])
                    if i + 1 < FCH:
                        psh = emit_psh(i + 1)
                    for m in range(DCH):
                        nc.tensor.matmul(
                            ps2[:, m, :], w2T[:, i, m * 128 : (m + 1) * 128],
                            hsb[:],
                            start=(i == 0), stop=(i == FCH - 1),
                            skip_group_check=True,
                        )
                for m in range(DCH):
                    nc.vector.scalar_tensor_tensor(
                        y2[:, m, hs], ps2[:, m, :], b2_sb[:, m : m + 1],
                        n1[:, m, hs], ALU.add, ALU.add,
                    )
        dump("y2", y2[:])
        _pw_stack.close()  # free woT/w1T/w2T before phase E

        # =================== Phase E: LN2 + output transpose ===============
        yf = pAct.tile([128, DCH, TOK], BF, tag="slotC")  # reuses ctxT slot
        with tc.tile_pool(name="pE2", bufs=2) as pE2, \
             tc.tile_pool(name="ps_bc2", bufs=2, space="PSUM") as ps_bc2:
            with tc.tile_pool(name="ps_st2", bufs=1, space="PSUM") as ps_st2:
                _emit_ln(tc, ps_bc2, ps_st2, pE2, y2, g2_sb, beta2_sb, yf)
            with tc.tile_pool(name="ps_tp2", bufs=2, space="PSUM") as ps_tp2:
                for t in range(TCH):
                    on = pE2.tile([128, D], F32, tag="on", name="on")
                    for j in range(DCH):
                        tp = ps_tp2.tile([128, 128], BF, tag="tp2", name="tp")
                        nc.tensor.transpose(tp[:],
                                            yf[:, j, t * 128 : (t + 1) * 128],
                                            ident_bf[:])
                        nc.vector.tensor_copy(on[:, j * 128 : (j + 1) * 128],
                                              tp[:])
                    nc.sync.dma_start(out_ap[t * 128 : (t + 1) * 128, :], on[:])


_CACHE = {}

DBG_SPECS = {
    "xT": ([128, DCH, TOK], BF), "QT": ([128, DCH, TOK], BF),
    "KhT0": ([128, KV], BF), "Vh0": ([128, KCH, VW], BF),
    "E0": ([128, TOK], BF),
    "ctxT": ([128, DCH, TOK], BF), "y1": ([128, DCH, TOK], BF),
    "n1": ([128, DCH, TOK], BF), "y2": ([128, DCH, TOK], BF),
}


def _build():
    if "nc" in _CACHE:
        return _CACHE["nc"]
    debug = os.environ.get("KERNEL_DEBUG", "0") == "1"
    nc = bacc.Bacc("TRN2", target_bir_lowering=False, debug=False,
                   num_devices=NCORES)
    t_in = {}
    t_in["x_shard"] = nc.dram_tensor("x_shard", [TOK, D], F32,
                                     kind="ExternalInput").ap()
    for name, shape in (
        ("wq", [D, D]), ("bq", [D]), ("wk", [D, D]), ("bk", [D]),
        ("wv", [D, D]), ("bv", [D]), ("wo", [D, D]), ("bo", [D]),
        ("w1", [DFF, D]), ("b1", [DFF]), ("w2", [D, DFF]), ("b2", [D]),
        ("g1", [D]), ("beta1", [D]), ("g2", [D]), ("beta2", [D]),
    ):
        t_in[name] = nc.dram_tensor(name, shape, F32, kind="ExternalInput").ap()
    t_out = {"out_shard": nc.dram_tensor("out_shard", [TOK, D], F32,
                                         kind="ExternalOutput").ap()}
    if debug:
        for name, (shape, dt) in DBG_SPECS.items():
            t_out["dbg_" + name] = nc.dram_tensor(
                "dbg_" + name, shape, dt, kind="ExternalOutput").ap()
    with tile.TileContext(nc) as tc:
        _emit_body(tc, t_in, t_out)
    nc.compile()
    _CACHE["nc"] = nc
    return nc


def _in_maps(inputs):
    f = lambda k: np.ascontiguousarray(np.asarray(inputs[k], dtype=np.float32))
    x = f("x")
    shared = {k: f(k) for k in inputs if k != "x"}
    maps = []
    for core in range(NCORES):
        g, r = divmod(core, GROUP)
        m = dict(shared)
        m["x_shard"] = np.ascontiguousarray(x[g, r * TOK : (r + 1) * TOK, :])
        maps.append(m)
    return maps


def kernel(**inputs):
    nc = _build()
    maps = _in_maps(inputs)
    res = run_bass_kernel_spmd(nc, maps, core_ids=list(range(NCORES)))
    shards = [res.results[i]["out_shard"] for i in range(NCORES)]
    out = np.concatenate(shards, axis=0).reshape(B, S, D)
    return out.astype(np.float32)
